# revision 13
# baseline (speedup 1.0000x reference)
"""Trainium2 Bass kernel for nn_Attention (LN -> QKV proj -> partial RoPE ->
null-KV prepend -> causal MQA attention -> out proj).

Sharding: 8 cores = 4 batches x 2 head-groups (8 heads each). Each core
computes its batch's LN/projections and its 8 heads' attention + partial
out-projection (through its W_out row-slice). Host sums the two head-group
partials per batch and stacks batches.

All compute ops keep uniform start-partitions (walrus checkSBSameStartPartition):
- k is projected twice (rows 0:64 and 64:128) so odd heads' QK matmuls run with
  lhsT/rhs both at base 64.
- rot projections are padded to pair layout so rope combines are base-aligned.
- the only cross-partition moves are SBUF->SBUF DMAs (odd-head attn-out rows,
  softmax-denominator row) and PE transposes.
"""

import sys

for _p in ("/opt/trn_rl_repo",):
    if _p not in sys.path:
        sys.path.insert(0, _p)

import numpy as np
import ml_dtypes

import concourse.bass as bass
import concourse.tile as tile
from concourse import bacc, mybir
from concourse import bass_utils

BF16 = ml_dtypes.bfloat16
F32 = np.float32

B, N, DIM = 4, 1024, 1024
HEADS, DH = 16, 64          # total heads; per-core 8
HPC = 8                     # heads per core
ROT = 32
NN = 2                      # null kv
EPS = 1e-5
P = 128
NEG = -1.0e38
SCALE = DH ** -0.5
NT = N // P                 # 8 i-tiles / D-chunks
IB = N // 512               # 2 i-blocks

dt = mybir.dt


def _chunks_for_block(b0):
    """j-tile chunks per i-block: lists of seq j-tile indices; 'T' = tail."""
    if b0 == 0:
        return [[0, 1, 2], [3, "T"]]
    return [[0, 1, 2], [3, 4, 5], [6, 7, "T"]]


def _build_program(mask_trivial):
    nc = bacc.Bacc("TRN2", target_bir_lowering=False, debug=False)

    f32, bf16 = dt.float32, dt.bfloat16
    AF = mybir.ActivationFunctionType
    OP = mybir.AluOpType

    d_x = nc.dram_tensor("x", [N, DIM], f32, kind="ExternalInput")
    d_wq = nc.dram_tensor("wq", [DIM, HPC * DH], bf16, kind="ExternalInput")
    # padded pair layout: per pair [even_rot(32), 0, odd_rot(32), 0]
    d_wqr = nc.dram_tensor("wqrot", [DIM, 4 * P], bf16, kind="ExternalInput")
    d_wkk = nc.dram_tensor("wkk", [DIM, P], bf16, kind="ExternalInput")   # [Wk|Wk]
    d_wv = nc.dram_tensor("wv", [DIM, DH], bf16, kind="ExternalInput")
    d_wkr = nc.dram_tensor("wkrot", [DIM, P], bf16, kind="ExternalInput")  # [krot,0,krot,0]
    d_wvr = nc.dram_tensor("wvrot", [DIM, DH], bf16, kind="ExternalInput")  # [vrot,0]
    d_wout = nc.dram_tensor("wout", [HPC * DH, DIM], bf16, kind="ExternalInput")
    d_cos = nc.dram_tensor("cosr", [P, N], bf16, kind="ExternalInput")
    d_sin = nc.dram_tensor("sinr", [P, N], bf16, kind="ExternalInput")
    d_tri = nc.dram_tensor("tri", [P, 5 * 512], bf16, kind="ExternalInput")
    d_ktail = nc.dram_tensor("ktail", [P, P], bf16, kind="ExternalInput")  # dup rows
    d_vtail = nc.dram_tensor("vtail", [P, DH + 1], bf16, kind="ExternalInput")
    d_qb = nc.dram_tensor("qbias", [P, 4], f32, kind="ExternalInput")
    d_qrb = nc.dram_tensor("qrotbias", [P, 4], f32, kind="ExternalInput")
    d_kb = nc.dram_tensor("kbias", [P, 1], f32, kind="ExternalInput")
    d_krb = nc.dram_tensor("krotbias", [P, 1], f32, kind="ExternalInput")
    d_vb = nc.dram_tensor("vbias", [DH, 1], f32, kind="ExternalInput")
    d_vrb = nc.dram_tensor("vrotbias", [DH, 1], f32, kind="ExternalInput")
    d_id = nc.dram_tensor("identm", [P, P], bf16, kind="ExternalInput")
    d_mb = None
    if not mask_trivial:
        d_mb = nc.dram_tensor("maskbias", [P, NT * 512], bf16, kind="ExternalInput")
    d_out = nc.dram_tensor("out", [N, DIM], f32, kind="ExternalOutput")

    with tile.TileContext(nc) as tc:
        from contextlib import ExitStack

        ctx = ExitStack()
        with ctx:
            consts = ctx.enter_context(tc.tile_pool(name="consts", bufs=1))
            persist = ctx.enter_context(tc.tile_pool(name="persist", bufs=1))

            # ---- persistent SBUF tensors ----
            wq_sb = consts.tile([P, NT * 512], bf16)       # 8 chunks x [128,512]
            wqr_sb = consts.tile([P, NT * 512], bf16)
            wkk_sb = consts.tile([P, NT * 128], bf16)
            wv_sb = consts.tile([P, NT * 64], bf16)
            wkr_sb = consts.tile([P, NT * 128], bf16)
            wvr_sb = consts.tile([P, NT * 64], bf16)
            wout_sb = consts.tile([P, 4 * DIM], bf16)      # 4 pair chunks
            cos_sb = consts.tile([P, N], bf16)
            sin_sb = consts.tile([P, N], bf16)
            tri_sb = consts.tile([P, 5 * 512], bf16)
            ktail_sb = consts.tile([P, P], bf16)
            vtail_sb = consts.tile([P, DH + 1], bf16)
            qb_sb = consts.tile([P, 4], f32)
            qrb_sb = consts.tile([P, 4], f32)
            kb_sb = consts.tile([P, 1], f32)
            krb_sb = consts.tile([P, 1], f32)
            vb_sb = consts.tile([DH, 1], f32)
            vrb_sb = consts.tile([DH, 1], f32)
            ident = consts.tile([P, P], bf16)
            mb_sb = None
            if not mask_trivial:
                mb_sb = consts.tile([P, NT * 512], bf16)

            xnT = persist.tile([P, NT * N], bf16)          # [D-chunk, i] chunks
            qp = persist.tile([P, 4 * N], bf16)            # q pairs [128, i]
            rotq = persist.tile([P, 4 * N], bf16)          # padded rot pairs
            kT = persist.tile([P, N], bf16)                # k duplicated rows
            rotk = persist.tile([P, N], bf16)
            vT = persist.tile([DH, N], bf16)
            rotv = persist.tile([DH, N], bf16)
            vext = persist.tile([P, 9 * (DH + 1)], bf16)   # row-major v + ones col
            ao = persist.tile([P, 4 * N], bf16)            # attn out pairs [128, i]
            shin = persist.tile([ROT, N], f32)             # recip row staging
            nc.vector.memset(shin[:], 1.0)

            for c in range(NT):
                sl = slice(c * P, (c + 1) * P)
                nc.gpsimd.dma_start(wq_sb[:, c * 512:(c + 1) * 512], d_wq.ap()[sl, :])
                nc.gpsimd.dma_start(wqr_sb[:, c * 512:(c + 1) * 512], d_wqr.ap()[sl, :])
                nc.gpsimd.dma_start(wkk_sb[:, c * 128:(c + 1) * 128], d_wkk.ap()[sl, :])
                nc.gpsimd.dma_start(wv_sb[:, c * 64:(c + 1) * 64], d_wv.ap()[sl, :])
                nc.gpsimd.dma_start(wkr_sb[:, c * 128:(c + 1) * 128], d_wkr.ap()[sl, :])
                nc.gpsimd.dma_start(wvr_sb[:, c * 64:(c + 1) * 64], d_wvr.ap()[sl, :])
            for p in range(4):
                nc.gpsimd.dma_start(wout_sb[:, p * DIM:(p + 1) * DIM],
                                  d_wout.ap()[p * P:(p + 1) * P, :])
            nc.gpsimd.dma_start(cos_sb[:], d_cos.ap()[:])
            nc.gpsimd.dma_start(sin_sb[:], d_sin.ap()[:])
            nc.gpsimd.dma_start(tri_sb[:], d_tri.ap()[:])
            nc.gpsimd.dma_start(ktail_sb[:], d_ktail.ap()[:])
            nc.gpsimd.dma_start(vtail_sb[:], d_vtail.ap()[:])
            nc.gpsimd.dma_start(qb_sb[:], d_qb.ap()[:])
            nc.gpsimd.dma_start(qrb_sb[:], d_qrb.ap()[:])
            nc.gpsimd.dma_start(kb_sb[:], d_kb.ap()[:])
            nc.gpsimd.dma_start(krb_sb[:], d_krb.ap()[:])
            nc.gpsimd.dma_start(vb_sb[:], d_vb.ap()[:])
            nc.gpsimd.dma_start(vrb_sb[:], d_vrb.ap()[:])
            nc.gpsimd.dma_start(ident[:], d_id.ap()[:])
            if not mask_trivial:
                nc.gpsimd.dma_start(mb_sb[:], d_mb.ap()[:])

            # ================= Phase 1: LN + transpose =================
            with tc.tile_pool(name="ph1sb", bufs=3) as ph1, \
                 tc.tile_pool(name="ph1st", bufs=8) as stp, \
                 tc.tile_pool(name="ph1ps", bufs=2, space="PSUM") as ps1:
                for t in range(NT):
                    xt = ph1.tile([P, DIM], f32, tag="x")
                    nc.gpsimd.dma_start(xt[:], d_x.ap()[t * P:(t + 1) * P, :])
                    rsum = stp.tile([P, 1], f32, tag="st")
                    nc.vector.tensor_reduce(rsum[:], xt[:],
                                            axis=mybir.AxisListType.X, op=OP.add)
                    sq = ph1.tile([P, DIM], bf16, tag="sq")
                    acc = stp.tile([P, 1], f32, tag="st")
                    nc.scalar.activation(sq[:], xt[:], AF.Square, accum_out=acc[:])
                    mean = stp.tile([P, 1], f32, tag="st")
                    nc.vector.tensor_scalar(out=mean[:], in0=rsum[:],
                                            scalar1=1.0 / DIM, scalar2=None,
                                            op0=OP.mult)
                    ex2 = stp.tile([P, 1], f32, tag="st")
                    nc.vector.tensor_scalar(out=ex2[:], in0=acc[:],
                                            scalar1=1.0 / DIM, scalar2=None,
                                            op0=OP.mult)
                    var = stp.tile([P, 1], f32, tag="st")
                    nc.vector.scalar_tensor_tensor(
                        out=var[:], in0=mean[:], scalar=-1.0, in1=mean[:],
                        op0=OP.mult, op1=OP.mult)
                    nc.vector.scalar_tensor_tensor(
                        out=var[:], in0=ex2[:], scalar=EPS, in1=var[:],
                        op0=OP.add, op1=OP.add)
                    lnv = stp.tile([P, 1], f32, tag="st")
                    nc.scalar.activation(lnv[:], var[:], AF.Ln)
                    rstd = stp.tile([P, 1], f32, tag="st")
                    nc.scalar.activation(rstd[:], lnv[:], AF.Exp, scale=-0.5)
                    negmr = stp.tile([P, 1], f32, tag="st")
                    nc.vector.scalar_tensor_tensor(
                        out=negmr[:], in0=mean[:], scalar=-1.0, in1=rstd[:],
                        op0=OP.mult, op1=OP.mult)
                    xn = ph1.tile([P, DIM], bf16, tag="xn")
                    nc.vector.tensor_scalar(out=xn[:], in0=xt[:],
                                            scalar1=rstd[:], scalar2=negmr[:],
                                            op0=OP.mult, op1=OP.add)
                    for g in range(2):  # chunk groups of 4
                        pst = ps1.tile([P, 512], bf16, tag="tp")
                        for c4 in range(4):
                            c = g * 4 + c4
                            nc.tensor.transpose(pst[:, c4 * P:(c4 + 1) * P],
                                                xn[:, c * P:(c + 1) * P], ident[:])
                        dest = xnT[:].rearrange("p (c i) -> p c i", c=NT)[
                            :, g * 4:(g + 1) * 4, t * P:(t + 1) * P]
                        src = pst[:].rearrange("p (c i) -> p c i", c=4)
                        nc.vector.tensor_copy(dest, src)

            # ================= Phase 2: projections + rope =================
            def mm_proj(ps2, w_sb, wwidth, col0, cols, ib, rows=P):
                """Accumulate [rows, 512] = W[:, col0:col0+cols]^T @ xn^T."""
                ps = ps2.tile([P, 512], f32, tag="proj")
                for c in range(NT):
                    nc.tensor.matmul(
                        ps[0:rows, :],
                        w_sb[:, c * wwidth + col0: c * wwidth + col0 + cols],
                        xnT[:, c * N + ib * 512: c * N + ib * 512 + 512],
                        start=(c == 0), stop=(c == NT - 1))
                return ps

            with tc.tile_pool(name="ph2ps", bufs=4, space="PSUM") as ps2, \
                 tc.tile_pool(name="ph2sb", bufs=2) as ph2:
                for p in range(4):
                    for ib in range(IB):
                        ps = mm_proj(ps2, wq_sb, 512, p * P, P, ib)
                        nc.vector.tensor_scalar(
                            out=qp[:, p * N + ib * 512: p * N + ib * 512 + 512],
                            in0=ps[:], scalar1=qb_sb[:, p:p + 1],
                            scalar2=None, op0=OP.add)
                        ps = mm_proj(ps2, wqr_sb, 512, p * P, P, ib)
                        nc.vector.tensor_scalar(
                            out=rotq[:, p * N + ib * 512: p * N + ib * 512 + 512],
                            in0=ps[:], scalar1=qrb_sb[:, p:p + 1],
                            scalar2=None, op0=OP.add)
                for ib in range(IB):
                    sl = slice(ib * 512, (ib + 1) * 512)
                    ps = mm_proj(ps2, wkk_sb, 128, 0, P, ib)
                    nc.vector.tensor_scalar(out=kT[:, sl], in0=ps[:],
                                            scalar1=kb_sb[:], scalar2=None,
                                            op0=OP.add)
                    ps = mm_proj(ps2, wkr_sb, 128, 0, P, ib)
                    nc.vector.tensor_scalar(out=rotk[:, sl], in0=ps[:],
                                            scalar1=krb_sb[:], scalar2=None,
                                            op0=OP.add)
                    ps = mm_proj(ps2, wv_sb, 64, 0, DH, ib, rows=DH)
                    nc.vector.tensor_scalar(out=vT[:, sl], in0=ps[0:DH, :],
                                            scalar1=vb_sb[:], scalar2=None,
                                            op0=OP.add)
                    ps = mm_proj(ps2, wvr_sb, 64, 0, DH, ib, rows=DH)
                    nc.vector.tensor_scalar(out=rotv[:, sl], in0=ps[0:DH, :],
                                            scalar1=vrb_sb[:], scalar2=None,
                                            op0=OP.add)

                # rope combine (all base-aligned; junk rows of rot* unused)
                nc.vector.tensor_tensor(out=rotk[:], in0=rotk[:],
                                        in1=sin_sb[:], op=OP.mult)
                nc.vector.tensor_tensor(out=rotv[0:ROT, :], in0=rotv[0:ROT, :],
                                        in1=sin_sb[0:ROT, :], op=OP.mult)
                for p in range(4):
                    sl = slice(p * N, (p + 1) * N)
                    nc.vector.tensor_tensor(out=rotq[:, sl], in0=rotq[:, sl],
                                            in1=sin_sb[:], op=OP.mult)
                for base in (0, DH):
                    rsl = slice(base, base + ROT)
                    for p in range(4):
                        csl = slice(p * N, (p + 1) * N)
                        nc.vector.tensor_tensor(
                            out=qp[rsl, csl], in0=qp[rsl, csl],
                            in1=cos_sb[rsl, :], op=OP.mult)
                        nc.vector.tensor_tensor(
                            out=qp[rsl, csl], in0=qp[rsl, csl],
                            in1=rotq[rsl, csl], op=OP.add)
                    nc.vector.tensor_tensor(out=kT[rsl, :], in0=kT[rsl, :],
                                            in1=cos_sb[rsl, :], op=OP.mult)
                    nc.vector.tensor_tensor(out=kT[rsl, :], in0=kT[rsl, :],
                                            in1=rotk[rsl, :], op=OP.add)
                nc.vector.tensor_tensor(out=vT[0:ROT, :], in0=vT[0:ROT, :],
                                        in1=cos_sb[0:ROT, :], op=OP.mult)
                nc.vector.tensor_tensor(out=vT[0:ROT, :], in0=vT[0:ROT, :],
                                        in1=rotv[0:ROT, :], op=OP.add)

            # ============ Phase 2b: v row-major + ones column ============
            with tc.tile_pool(name="vtp", bufs=2, space="PSUM") as vtp:
                for jj in range(NT):
                    pv = vtp.tile([P, DH], bf16, tag="vt")
                    nc.tensor.transpose(pv[:], vT[:, jj * P:(jj + 1) * P],
                                        ident[0:DH, 0:DH])
                    base = jj * (DH + 1)
                    nc.vector.tensor_copy(vext[:, base:base + DH], pv[:])
                    nc.vector.memset(vext[:, base + DH:base + DH + 1], 1.0)
                nc.vector.tensor_copy(vext[:, 8 * (DH + 1):9 * (DH + 1)],
                                      vtail_sb[:])

            # ================= Phase 3: attention =================
            with tc.tile_pool(name="simps", bufs=2, space="PSUM") as simps, \
                 tc.tile_pool(name="outps", bufs=2, space="PSUM") as outps, \
                 tc.tile_pool(name="atsb", bufs=3) as atsb, \
                 tc.tile_pool(name="nrm", bufs=2) as nrm:
                for h in range(HPC):
                    e = h % 2
                    hb = e * DH              # head base partition
                    pcol = h // 2
                    rsb = nrm.tile([P, N], f32, tag="rsb")
                    psos = []
                    for b0 in range(IB):
                        qh = qp[hb:hb + DH,
                                pcol * N + b0 * 512: pcol * N + b0 * 512 + 512]
                        chunks = _chunks_for_block(b0)
                        alljj = [jj for ch in chunks for jj in ch]
                        pso = outps.tile([P, 512], f32, tag="outT")
                        psos.append(pso)
                        first_av = True
                        for ch in chunks:
                            w = len(ch) * 512
                            pss = simps.tile([P, 1536], f32, tag="sim")
                            for idx, jj in enumerate(ch):
                                seg = pss[:, idx * 512:(idx + 1) * 512]
                                if jj == "T":
                                    nc.tensor.matmul(seg, ktail_sb[hb:hb + DH, :],
                                                     qh, start=True, stop=False)
                                    nc.tensor.matmul(
                                        seg, ident[:], tri_sb[:, 4 * 512:5 * 512],
                                        start=False, stop=True)
                                else:
                                    diag = jj >= 4 * b0
                                    extra = ((0 if mask_trivial else 1)
                                             + (1 if diag else 0))
                                    nc.tensor.matmul(
                                        seg, kT[hb:hb + DH, jj * P:(jj + 1) * P],
                                        qh, start=True, stop=(extra == 0))
                                    if diag:
                                        k = jj - 4 * b0
                                        extra -= 1
                                        nc.tensor.matmul(
                                            seg, ident[:],
                                            tri_sb[:, k * 512:(k + 1) * 512],
                                            start=False, stop=(extra == 0))
                                    if not mask_trivial:
                                        extra -= 1
                                        nc.tensor.matmul(
                                            seg, ident[:],
                                            mb_sb[:, jj * 512:(jj + 1) * 512],
                                            start=False, stop=(extra == 0))
                            at = atsb.tile([P, 1536], bf16, tag="at")
                            nc.scalar.activation(at[:, 0:w], pss[:, 0:w],
                                                 AF.Exp, scale=SCALE)
                            for idx, jj in enumerate(ch):
                                vjj = 8 if jj == "T" else jj
                                vcols = vext[:, vjj * (DH + 1):(vjj + 1) * (DH + 1)]
                                nc.tensor.matmul(
                                    pso[0:DH + 1, :], vcols,
                                    at[:, idx * 512:(idx + 1) * 512],
                                    start=first_av,
                                    stop=(jj == alljj[-1]))
                                first_av = False
                        # denominators -> rsb row 64 (same-base copy)
                        nc.vector.tensor_copy(
                            rsb[DH:DH + 1, b0 * 512:(b0 + 1) * 512],
                            pso[DH:DH + 1, :])
                    # recip = exp(-ln(s)) in place on the row
                    nc.scalar.activation(rsb[DH:DH + 1, :], rsb[DH:DH + 1, :],
                                         AF.Ln)
                    nc.scalar.activation(rsb[DH:DH + 1, :], rsb[DH:DH + 1, :],
                                         AF.Exp, scale=-1.0)
                    # move recip row to partition 0, broadcast to 64 rows
                    nc.gpsimd.dma_start(shin[0:1, :], rsb[DH:DH + 1, :])
                    bc = nrm.tile([DH, N], f32, tag="bc")
                    nc.vector.stream_shuffle(bc[0:ROT, :], shin[:], [0] * 32)
                    nc.gpsimd.dma_start(bc[ROT:DH, :], bc[0:ROT, :])
                    for b0 in range(IB):
                        osl = slice(pcol * N + b0 * 512, pcol * N + b0 * 512 + 512)
                        bsl = slice(b0 * 512, (b0 + 1) * 512)
                        if e == 0:
                            nc.vector.tensor_tensor(
                                out=ao[0:DH, osl], in0=psos[b0][0:DH, :],
                                in1=bc[:, bsl], op=OP.mult)
                        else:
                            tmp = nrm.tile([DH, 512], bf16, tag="tmp")
                            nc.vector.tensor_tensor(
                                out=tmp[:], in0=psos[b0][0:DH, :],
                                in1=bc[:, bsl], op=OP.mult)
                            nc.gpsimd.dma_start(ao[DH:P, osl], tmp[:])

            # ================= Phase 4: out projection =================
            with tc.tile_pool(name="opps", bufs=4, space="PSUM") as opps, \
                 tc.tile_pool(name="opsb", bufs=2) as opsb:
                for t in range(NT):
                    orow = opsb.tile([P, DIM], f32, tag="orow")
                    for nb in range(2):
                        ps = opps.tile([P, 512], f32, tag="op")
                        for p in range(4):
                            nc.tensor.matmul(
                                ps[:],
                                ao[:, p * N + t * P: p * N + t * P + 128],
                                wout_sb[:, p * DIM + nb * 512: p * DIM + nb * 512 + 512],
                                start=(p == 0), stop=(p == 3))
                        nc.vector.tensor_copy(orow[:, nb * 512:(nb + 1) * 512],
                                              ps[:])
                    nc.gpsimd.dma_start(d_out.ap()[t * P:(t + 1) * P, :], orow[:])

    nc.compile()
    return nc


_PROG_CACHE = {}


def _get_program(mask_trivial):
    key = bool(mask_trivial)
    if key not in _PROG_CACHE:
        _PROG_CACHE[key] = _build_program(key)
    return _PROG_CACHE[key]


def _rot_cols(Wb):
    """rotate_half on the output-dim axis of a [..., ROT] block:
    rot(t)[0:16] = -t[16:32]; rot(t)[16:32] = t[0:16]."""
    half = ROT // 2
    out = np.empty_like(Wb)
    out[..., 0:half] = -Wb[..., half:ROT]
    out[..., half:ROT] = Wb[..., 0:half]
    return out


def _host_prep(core, x, mask, freqs, ln_g, ln_b, W_q, W_kv, W_out, null_kv,
               mask_trivial):
    b, g = core // 2, core % 2
    heads = slice(g * HPC * DH, (g + 1) * HPC * DH)

    Wq_eff = (W_q * ln_g[:, None])[:, heads]            # [1024, 512]
    Wkv_eff = W_kv * ln_g[:, None]                      # [1024, 128]
    Wout_g = W_out[heads, :]                            # [512, 1024]
    bq = (ln_b @ W_q)[heads]                            # [512]
    bkv = ln_b @ W_kv                                   # [128]
    Wk, Wv = Wkv_eff[:, 0:DH], Wkv_eff[:, DH:2 * DH]
    bk, bv = bkv[0:DH], bkv[DH:2 * DH]

    # padded-pair rot weights for q: per pair [even_rot, 0, odd_rot, 0]
    wqrot = np.zeros((DIM, 4 * P), np.float64)
    qrotbias = np.zeros((P, 4), F32)
    for h in range(HPC):
        p, e = divmod(h, 2)
        blk = Wq_eff[:, h * DH: h * DH + ROT]
        wqrot[:, p * P + e * DH: p * P + e * DH + ROT] = _rot_cols(blk)
        qrotbias[e * DH:e * DH + ROT, p] = _rot_cols(bq[h * DH: h * DH + ROT])
    # duplicated k / krot (rows 0:64 == 64:128)
    wkk = np.concatenate([Wk, Wk], 1)
    kbias = np.concatenate([bk, bk]).reshape(P, 1).astype(F32)
    wkrot = np.zeros((DIM, P), np.float64)
    wkrot[:, 0:ROT] = _rot_cols(Wk[:, 0:ROT])
    wkrot[:, DH:DH + ROT] = wkrot[:, 0:ROT]
    krotbias = np.zeros((P, 1), F32)
    krotbias[0:ROT, 0] = _rot_cols(bk[0:ROT])
    krotbias[DH:DH + ROT, 0] = krotbias[0:ROT, 0]
    wvrot = np.zeros((DIM, DH), np.float64)
    wvrot[:, 0:ROT] = _rot_cols(Wv[:, 0:ROT])
    vrotbias = np.zeros((DH, 1), F32)
    vrotbias[0:ROT, 0] = _rot_cols(bv[0:ROT])

    qbias = np.zeros((P, 4), F32)
    for p in range(4):
        qbias[:, p] = bq[p * 128:(p + 1) * 128]

    f = np.asarray(freqs, np.float64)                   # [1024, 32]
    cosr = np.tile(np.cos(f).T, (4, 1))                 # [128, 1024]
    sinr = np.tile(np.sin(f).T, (4, 1))

    tri = np.zeros((P, 5 * 512), F32)
    pidx = np.arange(P)[:, None]
    il = np.arange(512)[None, :]
    for k in range(4):
        tri[:, k * 512:(k + 1) * 512] = np.where(il >= 128 * k + pidx, 0.0, NEG)
    tri[NN:, 4 * 512:5 * 512] = NEG                     # tail: rows >= 2 masked

    ktail = np.zeros((P, P), F32)
    nk = np.asarray(null_kv[0]).T                       # [64, 2]
    ktail[0:DH, 0:NN] = nk
    ktail[DH:P, 0:NN] = nk
    vtail = np.zeros((P, DH + 1), F32)
    vtail[0:NN, 0:DH] = np.asarray(null_kv[1])
    vtail[0:NN, DH] = 1.0

    im = {
        "x": np.ascontiguousarray(x[b], F32),
        "wq": Wq_eff.astype(BF16),
        "wqrot": wqrot.astype(BF16),
        "wkk": wkk.astype(BF16),
        "wv": np.ascontiguousarray(Wv).astype(BF16),
        "wkrot": wkrot.astype(BF16),
        "wvrot": wvrot.astype(BF16),
        "wout": np.ascontiguousarray(Wout_g).astype(BF16),
        "cosr": cosr.astype(BF16),
        "sinr": sinr.astype(BF16),
        "tri": tri.astype(BF16),
        "ktail": ktail.astype(BF16),
        "vtail": vtail.astype(BF16),
        "qbias": qbias,
        "qrotbias": qrotbias,
        "kbias": kbias,
        "krotbias": krotbias,
        "vbias": bv.reshape(DH, 1).astype(F32),
        "vrotbias": vrotbias,
        "identm": np.eye(P, dtype=BF16),
    }
    if not mask_trivial:
        mrow = np.where(np.asarray(mask[b]), 0.0, NEG)  # [1024]
        mb = np.zeros((P, NT * 512), F32)
        for jj in range(NT):
            mb[:, jj * 512:(jj + 1) * 512] = mrow[jj * P:(jj + 1) * P][:, None]
        im["maskbias"] = mb.astype(BF16)
    return im


def _run(x, mask, freqs, ln_g, ln_b, W_q, W_kv, W_out, null_kv, **spmd_kwargs):
    x = np.asarray(x, F32)
    mask = np.asarray(mask)
    freqs = np.asarray(freqs, F32)
    ln_g = np.asarray(ln_g, np.float64)
    ln_b = np.asarray(ln_b, np.float64)
    W_q = np.asarray(W_q, np.float64)
    W_kv = np.asarray(W_kv, np.float64)
    W_out = np.asarray(W_out, np.float64)
    null_kv = np.asarray(null_kv, F32)

    mask_trivial = bool(mask.all())
    nc = _get_program(mask_trivial)
    in_maps = [
        _host_prep(c, x, mask, freqs, ln_g, ln_b, W_q, W_kv, W_out, null_kv,
                   mask_trivial)
        for c in range(8)
    ]
    res = bass_utils.run_bass_kernel_spmd(nc, in_maps, list(range(8)),
                                          **spmd_kwargs)
    out = np.empty((B, N, DIM), F32)
    for b in range(B):
        out[b] = res.results[2 * b]["out"] + res.results[2 * b + 1]["out"]
    return out, res


def kernel(x, mask, freqs, ln_g, ln_b, W_q, W_kv, W_out, null_kv):
    out, _ = _run(x, mask, freqs, ln_g, ln_b, W_q, W_kv, W_out, null_kv)
    return out
